# revision 39
# baseline (speedup 1.0000x reference)
"""Trainium2 Bass kernel for nn_CTNet (dense GNN with CT-rewire + mincut pool).

Strategy: data-parallel over the 16 graphs -> 2 graphs per NeuronCore (8 cores).
Everything stays in SBUF per graph:
  - dense adjT (transposed adjacency, counts) built on TensorE from one-hot
    compares (DVE is_equal vs an iota row) + PSUM-accumulated matmuls,
  - cdist via one augmented fp32 matmul per tile (D^2 = -2*S@S^T + sq_i + sq_j),
  - CT-rewire normalization folded into the next matmuls (adj2 never
    materialized; only new_adjT = D * adjT and a rinv vector),
  - mincut pool / ortho losses reduced on-chip to 9 per-graph scalars,
  - final combine of per-graph scalars on host (cross-shard mean).

Self-contained: hardcodes all shapes from the problem spec.
"""

import numpy as np

B, N, E = 16, 1024, 262144
NCORES = 8
GPC = B // NCORES          # graphs per core = 2
CIN, HID, K1, K2, COUT = 64, 32, 100, 16, 10
P = 128
NT = N // P                # 8 node tiles per graph
EPS = 1e-15
EPSM = 1e-8                # (unused) diagonal clamp for D^2
EPS_POS = 1.2e-5           # sqrt-input shift; corrected in sumAdjD2 on host
NSCAL = 16

_CACHE = {}


def _build_nc(cap_chunks, repeat=1):
    import concourse.bass as bass
    import concourse.tile as tile
    from concourse import bacc, mybir
    from concourse.masks import make_identity

    fp32 = mybir.dt.float32
    bf16 = mybir.dt.bfloat16
    AL = mybir.AluOpType
    AF = mybir.ActivationFunctionType

    CAP = cap_chunks * P
    NCH = GPC * NT * NT * cap_chunks   # edge chunks per core

    nc = bacc.Bacc("TRN2", target_bir_lowering=False, debug=False)

    # ---- I/O ----
    xt_d = nc.dram_tensor("xt", [P, GPC * NT * CIN], fp32, kind="ExternalInput")
    esrc_d = nc.dram_tensor("esrc", [P, NCH], bf16, kind="ExternalInput")
    edst_d = nc.dram_tensor("edst", [P, NCH], bf16, kind="ExternalInput")
    iota_d = nc.dram_tensor("iota", [P, 4 * P], bf16, kind="ExternalInput")
    w_d = {}
    for name, shape in [
        ("W1a", [CIN + 1, HID]), ("P1a", [HID + 1, K1]), ("P2a", [HID + 1, K2]),
        ("C1a", [HID + 1, HID]), ("C1r", [HID, HID]),
        ("C2a", [HID + 1, HID]), ("C2r", [HID, HID]),
        ("L2a", [HID + 1, HID]), ("L3a", [HID + 1, COUT]),
    ]:
        w_d[name] = nc.dram_tensor(name, shape, fp32, kind="ExternalInput")
    logits_d = nc.dram_tensor("logits", [GPC, COUT], fp32, kind="ExternalOutput")
    scal_d = nc.dram_tensor("scal", [GPC, NSCAL], fp32, kind="ExternalOutput")

    with tile.TileContext(nc) as tc:
        with (
            tc.tile_pool(name="const", bufs=1) as cpool,
            tc.tile_pool(name="adjp", bufs=2) as adjpool,
            tc.tile_pool(name="work", bufs=2) as wkpool,
            tc.tile_pool(name="uv", bufs=4) as uvpool,
            tc.tile_pool(name="small", bufs=3) as smpool,
            tc.tile_pool(name="rowp", bufs=2) as rowpool,
            tc.tile_pool(name="dram", bufs=2, space="DRAM") as dpool,
            tc.tile_pool(name="psA", bufs=2, space="PSUM") as psA,
            tc.tile_pool(name="psB", bufs=2, space="PSUM") as psB,
            tc.tile_pool(name="psC", bufs=2, space="PSUM") as psC,
            tc.tile_pool(name="psD", bufs=2, space="PSUM") as psD,
        ):
            # ---- constants ----
            IOTA4 = cpool.tile([P, 4 * P], bf16, tag="iota")
            nc.sync.dma_start(out=IOTA4[:], in_=iota_d[:, :])
            IDENT = cpool.tile([P, P], fp32, tag="ident")
            make_identity(nc, IDENT[:])
            ones_col = cpool.tile([P, 1], fp32, tag="onescol")
            nc.gpsimd.memset(ones_col[:], 1.0)
            ones_colb = cpool.tile([P, 1], bf16, tag="onescolb")
            nc.gpsimd.memset(ones_colb[:], 1.0)
            ones_row = cpool.tile([1, P], fp32, tag="onesrow")
            nc.gpsimd.memset(ones_row[:], 1.0)
            ones_rowb = cpool.tile([1, P], bf16, tag="onesrowb")
            nc.gpsimd.memset(ones_rowb[:], 1.0)

            W = {}
            for name, t in w_d.items():
                shape = [t.shape[0], t.shape[1]]
                W[name] = cpool.tile(shape, fp32, tag=f"w_{name}", name=f"w_{name}")
                nc.sync.dma_start(out=W[name][:], in_=t[:, :])

            I16f = cpool.tile([K2, K2], fp32, tag="i16")
            make_identity(nc, I16f[:])
            mask16 = cpool.tile([K2, K2], fp32, tag="mask16")
            nc.vector.tensor_scalar(
                out=mask16[:], in0=I16f[:], scalar1=-1.0, scalar2=1.0,
                op0=AL.mult, op1=AL.add)

            esrc = cpool.tile([P, NCH], bf16, tag="esrc")
            nc.sync.dma_start(out=esrc[:], in_=esrc_d[:, :])
            edst = cpool.tile([P, NCH], bf16, tag="edst")
            nc.sync.dma_start(out=edst[:], in_=edst_d[:, :])
            xt_sb = cpool.tile([P, GPC * NT * CIN], fp32, tag="xt")
            nc.sync.dma_start(out=xt_sb[:], in_=xt_d[:, :])

            def _body():
                for g in range(GPC):
                    _emit_graph(nc, tc, mybir, AL, AF, g, cap_chunks,
                                IOTA4, IDENT, ones_col, ones_colb, ones_row, ones_rowb, W, mask16,
                                esrc, edst, xt_sb,
                                adjpool, wkpool, uvpool, smpool, rowpool, dpool,
                                psA, psB, psC, psD,
                                logits_d, scal_d)

            if repeat == 1:
                _body()
            else:
                with tc.For_i(0, repeat, 1):
                    _body()

    nc.compile()
    return nc


def _emit_graph(nc, tc, mybir, AL, AF, g, cap_chunks,
                IOTA4, IDENT, ones_col, ones_colb, ones_row, ones_rowb, W, mask16,
                esrc, edst, xt_sb,
                adjpool, wkpool, uvpool, smpool, rowpool, dpool,
                psA, psB, psC, psD,
                logits_d, scal_d):
    fp32 = mybir.dt.float32
    bf16 = mybir.dt.bfloat16
    H = 512  # matmul free-dim slice

    # ---------------- persistent per-graph tiles ----------------
    adjT = adjpool.tile([P, NT * N], bf16, tag="adjT")          # 16KB/part
    xTa = wkpool.tile([CIN + 1, N], fp32, tag="xTa", bufs=1)
    hTa = wkpool.tile([HID + 1, N], fp32, tag="hTa")
    h1Ta = wkpool.tile([HID + 1, N], fp32, tag="h1Ta")
    s1m = wkpool.tile([P, NT * K1], fp32, tag="s1m")
    s2m = wkpool.tile([P, NT * K2], fp32, tag="s2m")
    s2p = wkpool.tile([P, NT * K2], bf16, tag="s2p")
    hp = wkpool.tile([P, NT * HID], bf16, tag="hp")
    h1nm = wkpool.tile([P, NT * HID], fp32, tag="h1nm")
    vnm = wkpool.tile([P, NT * K2], fp32, tag="vnm")
    rhs_s = wkpool.tile([K1, N], fp32, tag="rhs_s")
    R32 = wkpool.tile([HID, N], fp32, tag="R32", bufs=1)
    aTa = wkpool.tile([HID + 1, N], fp32, tag="aTa", bufs=1)
    uT = wkpool.tile([K2, N], fp32, tag="uT", bufs=1)

    sq1c = smpool.tile([P, NT], fp32, tag="sq1c")
    sqdc = smpool.tile([P, NT], fp32, tag="sqdc")
    sq2c = smpool.tile([P, NT], fp32, tag="sq2c")
    dinc = smpool.tile([P, NT], fp32, tag="dinc")
    dinc2 = smpool.tile([P, 2 * NT], fp32, tag="dinc2")
    rinvc = smpool.tile([P, NT], fp32, tag="rinvc")
    sadj_parts = smpool.tile([P, 2 * NT], fp32, tag="sadjp")
    SCAL = smpool.tile([P, NSCAL], fp32, tag="SCAL")
    nc.vector.memset(SCAL[:], 0.0)

    rows = {}
    for nm in ("dflat_row", "r_row", "rinv_row", "ar_row", "sq_row", "sq2_row", "sqd_row"):
        rows[nm] = rowpool.tile([1, N], fp32, tag=nm, name=nm)

    # ---------------- A: transpose x tiles -> xTa ----------------
    for t in range(NT):
        ps = psD.tile([CIN, P], fp32, space="PSUM", tag="ps_sm")
        nc.tensor.transpose(
            out=ps[:], in_=xt_sb[:, (g * NT + t) * CIN:(g * NT + t + 1) * CIN],
            identity=IDENT[:])
        nc.scalar.copy(out=xTa[0:CIN, t * P:(t + 1) * P], in_=ps[:])
    nc.gpsimd.memset(xTa[CIN:CIN + 1, :], 1.0)

    # ---------------- B: hTa = W1a.T @ xTa ----------------
    for h in range(2):
        ps_h = psC.tile([HID, H], fp32, space="PSUM", tag="ps_wide", name="ps_h")
        nc.tensor.matmul(out=ps_h[:], lhsT=W["W1a"][:],
                         rhs=xTa[:, h * H:(h + 1) * H], start=True, stop=True)
        nc.scalar.copy(out=hTa[0:HID, h * H:(h + 1) * H], in_=ps_h[:])
    nc.gpsimd.memset(hTa[HID:HID + 1, :], 1.0)

    # ---------------- C: s1 node-major, softmax, sq1 (packed) ----------------
    e1 = uvpool.tile([P, NT * K1], fp32, tag="e1", bufs=1)
    for hh in range(2):
        ps = psC.tile([P, 4 * P], fp32, space="PSUM", tag="ps_wide",
                      name="ps_s1")
        for t4 in range(4):
            t = hh * 4 + t4
            nc.tensor.matmul(out=ps[:, t4 * P:t4 * P + K1],
                             lhsT=hTa[:, t * P:(t + 1) * P],
                             rhs=W["P1a"][:], start=True, stop=True)
        nc.scalar.activation(
            out=e1[:, hh * 4 * K1:(hh + 1) * 4 * K1].rearrange(
                "p (a b) -> p a b", a=4),
            in_=ps[:].rearrange("p (a b) -> p a b", a=4)[:, :, 0:K1],
            func=AF.Exp)
    esum8 = smpool.tile([P, NT], fp32, tag="esum8")
    nc.vector.tensor_reduce(out=esum8[:],
                            in_=e1[:].rearrange("p (a b) -> p a b", a=NT),
                            axis=mybir.AxisListType.X, op=AL.add)
    einv8 = smpool.tile([P, NT], fp32, tag="einv8")
    nc.vector.reciprocal(out=einv8[:], in_=esum8[:])
    nc.vector.tensor_tensor(
        out=s1m[:].rearrange("p (a b) -> p a b", a=NT),
        in0=e1[:].rearrange("p (a b) -> p a b", a=NT),
        in1=einv8[:, :, None].to_broadcast([P, NT, K1]),
        op=AL.mult)
    scr1 = uvpool.tile([P, NT * K1], fp32, tag="scr_k1", bufs=1)
    nc.vector.tensor_tensor(out=scr1[:], in0=s1m[:], in1=s1m[:], op=AL.mult)
    nc.vector.tensor_reduce(out=sq1c[:],
                            in_=scr1[:].rearrange("p (a b) -> p a b", a=NT),
                            axis=mybir.AxisListType.X, op=AL.add)
    # sq_row via DRAM round trip: sq1c [128, NT] -> dram (t*128+p order) -> row
    dsq = dpool.tile([N], fp32, tag="dsq", name="dsq")
    nc.sync.dma_start(out=dsq[:].rearrange("(t p) -> p t", p=P), in_=sq1c[:])
    nc.sync.dma_start(out=rows["sq_row"][:], in_=dsq[:][None, :])

    # ------- D: centered s1mT in bf16 (lhs=-2*delta, rhs=delta) + sqd -------
    for t in range(NT):
        ps = psD.tile([K1, P], fp32, space="PSUM", tag="ps_sm")
        nc.tensor.transpose(out=ps[:], in_=s1m[:, t * K1:(t + 1) * K1],
                            identity=IDENT[:])
        nc.scalar.copy(out=rhs_s[:, t * P:(t + 1) * P], in_=ps[:])
    # per-feature mean over nodes (distances are translation invariant)
    mu = smpool.tile([K1, 1], fp32, tag="mu")
    nc.vector.tensor_reduce(out=mu[:], in_=rhs_s[:],
                            axis=mybir.AxisListType.X, op=AL.add)
    nc.vector.tensor_scalar_mul(out=mu[:], in0=mu[:], scalar1=1.0 / N)
    rhs_b = wkpool.tile([K1, N], bf16, tag="rhs_b")
    nc.vector.tensor_scalar(out=rhs_b[:], in0=rhs_s[:], scalar1=mu[:],
                            scalar2=1.0, op0=AL.subtract, op1=AL.mult)
    lhs_b = wkpool.tile([K1, N], bf16, tag="lhs_b")
    nc.vector.tensor_scalar(out=lhs_b[:], in0=rhs_s[:], scalar1=mu[:],
                            scalar2=-2.0, op0=AL.subtract, op1=AL.mult)
    d2b = wkpool.tile([K1, N], bf16, tag="d2b")
    nc.vector.tensor_tensor(out=d2b[:], in0=rhs_b[:], in1=rhs_b[:], op=AL.mult)
    for h in range(2):
        ps_sq = psC.tile([1, H], fp32, space="PSUM", tag="ps_wide", name="ps_sq")
        nc.tensor.matmul(out=ps_sq[:], lhsT=ones_colb[0:K1, :],
                         rhs=d2b[:, h * H:(h + 1) * H], start=True, stop=True)
        nc.scalar.copy(out=rows["sqd_row"][0:1, h * H:(h + 1) * H], in_=ps_sq[:])
    sqd_rb = rowpool.tile([1, N], bf16, tag="sqd_rb", name="sqd_rb")
    nc.vector.tensor_copy(out=sqd_rb[:], in_=rows["sqd_row"][:])
    dqd = dpool.tile([N], fp32, tag="dqd", name="dqd")
    nc.sync.dma_start(out=dqd[:][None, :], in_=rows["sqd_row"][:])
    nc.sync.dma_start(out=sqdc[:], in_=dqd[:].rearrange("(t p) -> p t", p=P))
    nc.vector.tensor_scalar(out=sqdc[:], in0=sqdc[:], scalar1=EPS_POS,
                            scalar2=0.0, op0=AL.add, op1=AL.add)


    # ---------------- E: adjT build (quad-batched one-hot compares) ----------
    CPG = 4 * cap_chunks             # chunks per (jt, hu) group
    NQ = (CPG + 3) // 4              # quads per group
    for jt in range(NT):
        for hu in range(2):
            ps_adj = psA.tile([P, H], fp32, space="PSUM", tag="ps_adj")
            base = (((g * NT + jt) * NT + hu * 4) * cap_chunks)
            for q in range(NQ):
                k0 = q * 4
                kw = min(4, CPG - k0)
                U4 = uvpool.tile([P, 4 * P], bf16, tag="U", bufs=6)
                V4 = uvpool.tile([P, 4 * P], bf16, tag="V", bufs=6)
                nc.vector.tensor_tensor(
                    out=U4[:, 0:kw * P].rearrange("p (a b) -> p a b", a=kw),
                    in0=edst[:, base + k0:base + k0 + kw].to_broadcast([P, kw, P]),
                    in1=IOTA4[:, 0:kw * P].rearrange("p (a b) -> p a b", a=kw),
                    op=AL.is_equal)
                nc.vector.tensor_tensor(
                    out=V4[:, 0:kw * P].rearrange("p (a b) -> p a b", a=kw),
                    in0=esrc[:, base + k0:base + k0 + kw].to_broadcast([P, kw, P]),
                    in1=IOTA4[:, 0:kw * P].rearrange("p (a b) -> p a b", a=kw),
                    op=AL.is_equal)
                for k in range(kw):
                    idx = k0 + k
                    u4 = idx // cap_chunks
                    c = idx % cap_chunks
                    nc.tensor.matmul(out=ps_adj[:, u4 * P:(u4 + 1) * P],
                                     lhsT=U4[:, k * P:(k + 1) * P],
                                     rhs=V4[:, k * P:(k + 1) * P],
                                     start=(c == 0), stop=(c == cap_chunks - 1))
            nc.scalar.activation(
                out=adjT[:, jt * N + hu * H: jt * N + (hu + 1) * H],
                in_=ps_adj[:], func=AF.Copy,
                accum_out=dinc2[:, jt * 2 + hu: jt * 2 + hu + 1])
    nc.vector.tensor_tensor(out=dinc[:], in0=dinc2[:, 0:2 * NT:2],
                            in1=dinc2[:, 1:2 * NT:2], op=AL.add)

    # ---------------- F: d_flat (col sums of adjT) ----------------
    for h in range(2):
        ps_df = psC.tile([1, H], fp32, space="PSUM", tag="ps_wide", name="ps_df")
        for jt in range(NT):
            nc.tensor.matmul(out=ps_df[:], lhsT=ones_colb[:],
                             rhs=adjT[:, jt * N + h * H: jt * N + (h + 1) * H],
                             start=(jt == 0), stop=(jt == NT - 1))
        nc.scalar.copy(out=rows["dflat_row"][0:1, h * H:(h + 1) * H], in_=ps_df[:])

    # ---------------- G: cdist, mask, r, rinv ----------------
    for jt in range(NT):
        for h in range(2):
            ps_d2 = psB.tile([P, H], fp32, space="PSUM", tag="ps_d2")
            nc.tensor.matmul(out=ps_d2[:],
                             lhsT=lhs_b[:, jt * P:(jt + 1) * P],
                             rhs=rhs_b[:, h * H:(h + 1) * H],
                             start=True, stop=False)
            nc.tensor.matmul(out=ps_d2[:],
                             lhsT=ones_rowb[0:1, 0:P],
                             rhs=sqd_rb[0:1, h * H:(h + 1) * H],
                             start=False, stop=True)
            t2 = uvpool.tile([P, H], bf16, tag="t2")
            nc.scalar.activation(out=t2[:], in_=ps_d2[:], func=AF.Sqrt,
                                 bias=sqdc[:, jt:jt + 1], scale=1.0)
            if jt * P >= h * H and jt * P < (h + 1) * H:
                doff = jt * P - h * H
                nc.gpsimd.affine_select(
                    out=t2[:, doff:doff + P], in_=t2[:, doff:doff + P],
                    compare_op=AL.not_equal, fill=0.0,
                    base=0, pattern=[[-1, P]], channel_multiplier=1)
            asl = adjT[:, jt * N + h * H: jt * N + (h + 1) * H]
            nc.vector.tensor_tensor(out=asl, in0=t2[:], in1=asl, op=AL.mult)
            scr = uvpool.tile([P, H], bf16, tag="scr_h")
            nc.vector.scalar_tensor_tensor(
                out=scr[:], in0=asl, scalar=1.0, in1=t2[:],
                op0=AL.bypass, op1=AL.mult,
                accum_out=sadj_parts[:, jt * 2 + h: jt * 2 + h + 1])
    for h in range(2):
        ps_r = psC.tile([1, H], fp32, space="PSUM", tag="ps_wide", name="ps_r")
        for jt in range(NT):
            nc.tensor.matmul(out=ps_r[:], lhsT=ones_colb[:],
                             rhs=adjT[:, jt * N + h * H: jt * N + (h + 1) * H],
                             start=(jt == 0), stop=(jt == NT - 1))
        nc.scalar.copy(out=rows["r_row"][0:1, h * H:(h + 1) * H], in_=ps_r[:])
    nc.scalar.activation(out=rows["r_row"][:], in_=rows["r_row"][:],
                         func=AF.Sqrt)
    nc.vector.tensor_scalar(out=rows["r_row"][:], in0=rows["r_row"][:],
                            scalar1=EPS, scalar2=0.0, op0=AL.add, op1=AL.add)
    nc.vector.reciprocal(out=rows["rinv_row"][:], in_=rows["r_row"][:])
    # replicate rinv into R32 [32, N] and column form rinvc [128, 8]
    for h in range(2):
        ps_R = psC.tile([HID, H], fp32, space="PSUM", tag="ps_wide", name="ps_R")
        nc.tensor.matmul(out=ps_R[:],
                         lhsT=ones_row[0:1, 0:HID],
                         rhs=rows["rinv_row"][0:1, h * H:(h + 1) * H],
                         start=True, stop=True)
        nc.scalar.copy(out=R32[:, h * H:(h + 1) * H], in_=ps_R[:])
    drv = dpool.tile([N], fp32, tag="drv", name="drv")
    nc.sync.dma_start(out=drv[:][None, :], in_=rows["rinv_row"][:])
    nc.sync.dma_start(out=rinvc[:], in_=drv[:].rearrange("(t p) -> p t", p=P))
    rinvcb = smpool.tile([P, NT], bf16, tag="rinvcb")
    nc.gpsimd.dma_start(out=rinvcb[:], in_=drv[:].rearrange("(t p) -> p t", p=P))

    # ---------------- H: scalar reductions (part 1) ----------------
    nc.vector.scalar_tensor_tensor(
        out=rows["ar_row"][:], in0=rows["dflat_row"][:], scalar=1.0,
        in1=rows["sq_row"][:],
        op0=AL.bypass, op1=AL.mult, accum_out=SCAL[0:1, 0:1])
    scr8b = smpool.tile([P, NT], fp32, tag="scr8b")
    nc.vector.scalar_tensor_tensor(
        out=scr8b[:], in0=dinc[:], scalar=1.0, in1=sq1c[:],
        op0=AL.bypass, op1=AL.mult, accum_out=SCAL[:, 1:2])
    nc.vector.tensor_reduce(out=SCAL[:, 2:3], in_=sadj_parts[:],
                            axis=mybir.AxisListType.X, op=AL.add)
    nc.vector.tensor_reduce(out=SCAL[:, 3:4], in_=sq1c[:],
                            axis=mybir.AxisListType.X, op=AL.add)
    # ss1 = s1m^T s1m, F1sq
    ps_ss = psD.tile([K1, K1], fp32, space="PSUM", tag="ps_sm")
    for t in range(NT):
        nc.tensor.matmul(out=ps_ss[:], lhsT=s1m[:, t * K1:(t + 1) * K1],
                         rhs=s1m[:, t * K1:(t + 1) * K1],
                         start=(t == 0), stop=(t == NT - 1))
    ss1_sb = uvpool.tile([K1, K1], fp32, tag="ss1")
    nc.scalar.copy(out=ss1_sb[:], in_=ps_ss[:])
    ssscr = uvpool.tile([K1, K1], fp32, tag="ssscr")
    nc.vector.scalar_tensor_tensor(
        out=ssscr[:], in0=ss1_sb[:], scalar=1.0, in1=ss1_sb[:],
        op0=AL.bypass, op1=AL.mult, accum_out=SCAL[0:K1, 4:5])

    # ---------------- I: conv1 ----------------
    for t in range(NT):
        ps = psD.tile([P, HID], fp32, space="PSUM", tag="ps_sm")
        nc.tensor.transpose(out=ps[:], in_=hTa[0:HID, t * P:(t + 1) * P],
                            identity=IDENT[0:HID, 0:HID])
        nc.vector.tensor_scalar_mul(out=hp[:, t * HID:(t + 1) * HID],
                                    in0=ps[:], scalar1=rinvc[:, t:t + 1])
    for h in range(2):
        ps_aT = psC.tile([HID, H], fp32, space="PSUM", tag="ps_wide", name="ps_aT")
        for jt in range(NT):
            nc.tensor.matmul(out=ps_aT[:],
                             lhsT=hp[:, jt * HID:(jt + 1) * HID],
                             rhs=adjT[:, jt * N + h * H: jt * N + (h + 1) * H],
                             start=(jt == 0), stop=(jt == NT - 1))
        nc.vector.tensor_tensor(out=aTa[0:HID, h * H:(h + 1) * H], in0=ps_aT[:],
                                in1=R32[:, h * H:(h + 1) * H], op=AL.mult)
    nc.gpsimd.memset(aTa[HID:HID + 1, :], 1.0)
    for h in range(2):
        ps_h1 = psC.tile([HID, H], fp32, space="PSUM", tag="ps_wide", name="ps_h1")
        nc.tensor.matmul(out=ps_h1[:], lhsT=W["C1a"][:],
                         rhs=aTa[:, h * H:(h + 1) * H], start=True, stop=False)
        nc.tensor.matmul(out=ps_h1[:], lhsT=W["C1r"][:],
                         rhs=hTa[0:HID, h * H:(h + 1) * H],
                         start=False, stop=True)
        nc.scalar.copy(out=h1Ta[0:HID, h * H:(h + 1) * H], in_=ps_h1[:])
    nc.gpsimd.memset(h1Ta[HID:HID + 1, :], 1.0)
    for t in range(NT):
        ps = psD.tile([P, HID], fp32, space="PSUM", tag="ps_sm")
        nc.tensor.transpose(out=ps[:], in_=h1Ta[0:HID, t * P:(t + 1) * P],
                            identity=IDENT[0:HID, 0:HID])
        nc.scalar.copy(out=h1nm[:, t * HID:(t + 1) * HID], in_=ps[:])

    # ---------------- J: s2 softmax (packed) ----------------
    e2 = uvpool.tile([P, NT * K2], fp32, tag="e2", bufs=1)
    for hh in range(2):
        ps = psC.tile([P, 4 * P], fp32, space="PSUM", tag="ps_wide",
                      name="ps_s2")
        for t4 in range(4):
            t = hh * 4 + t4
            nc.tensor.matmul(out=ps[:, t4 * P:t4 * P + K2],
                             lhsT=h1Ta[:, t * P:(t + 1) * P],
                             rhs=W["P2a"][:], start=True, stop=True)
        nc.scalar.activation(
            out=e2[:, hh * 4 * K2:(hh + 1) * 4 * K2].rearrange(
                "p (a b) -> p a b", a=4),
            in_=ps[:].rearrange("p (a b) -> p a b", a=4)[:, :, 0:K2],
            func=AF.Exp)
    esum8b = smpool.tile([P, NT], fp32, tag="esum8b")
    nc.vector.tensor_reduce(out=esum8b[:],
                            in_=e2[:].rearrange("p (a b) -> p a b", a=NT),
                            axis=mybir.AxisListType.X, op=AL.add)
    einv8b = smpool.tile([P, NT], fp32, tag="einv8b")
    nc.vector.reciprocal(out=einv8b[:], in_=esum8b[:])
    nc.vector.tensor_tensor(
        out=s2m[:].rearrange("p (a b) -> p a b", a=NT),
        in0=e2[:].rearrange("p (a b) -> p a b", a=NT),
        in1=einv8b[:, :, None].to_broadcast([P, NT, K2]),
        op=AL.mult)
    scr2 = uvpool.tile([P, NT * K2], fp32, tag="scr_k2", bufs=1)
    nc.vector.tensor_tensor(out=scr2[:], in0=s2m[:], in1=s2m[:], op=AL.mult)
    nc.vector.tensor_reduce(out=sq2c[:],
                            in_=scr2[:].rearrange("p (a b) -> p a b", a=NT),
                            axis=mybir.AxisListType.X, op=AL.add)
    nc.vector.tensor_tensor(
        out=s2p[:].rearrange("p (a b) -> p a b", a=NT),
        in0=s2m[:].rearrange("p (a b) -> p a b", a=NT),
        in1=rinvc[:, :, None].to_broadcast([P, NT, K2]),
        op=AL.mult)

    # ---------------- K: u = adj2 @ s2m, num/den, ss2, out ----------------
    for h in range(2):
        ps_uT = psC.tile([K2, H], fp32, space="PSUM", tag="ps_wide", name="ps_uT")
        for jt in range(NT):
            nc.tensor.matmul(out=ps_uT[:],
                             lhsT=s2p[:, jt * K2:(jt + 1) * K2],
                             rhs=adjT[:, jt * N + h * H: jt * N + (h + 1) * H],
                             start=(jt == 0), stop=(jt == NT - 1))
        nc.vector.tensor_tensor(out=uT[:, h * H:(h + 1) * H], in0=ps_uT[:],
                                in1=R32[0:K2, h * H:(h + 1) * H], op=AL.mult)
    for t in range(NT):
        ps = psD.tile([P, K2], fp32, space="PSUM", tag="ps_sm")
        nc.tensor.transpose(out=ps[:], in_=uT[:, t * P:(t + 1) * P],
                            identity=IDENT[0:K2, 0:K2])
        nc.scalar.copy(out=vnm[:, t * K2:(t + 1) * K2], in_=ps[:])
    scrn = uvpool.tile([P, NT * K2], fp32, tag="scr_k2", bufs=1)
    nc.vector.scalar_tensor_tensor(
        out=scrn[:], in0=s2m[:], scalar=1.0, in1=vnm[:],
        op0=AL.bypass, op1=AL.mult, accum_out=SCAL[:, 7:8])
    # ar = adj2 row sums
    for h in range(2):
        ps_ar = psC.tile([1, H], fp32, space="PSUM", tag="ps_wide", name="ps_ar")
        for jt in range(NT):
            nc.tensor.matmul(out=ps_ar[:],
                             lhsT=rinvcb[:, jt:jt + 1],
                             rhs=adjT[:, jt * N + h * H: jt * N + (h + 1) * H],
                             start=(jt == 0), stop=(jt == NT - 1))
        nc.scalar.copy(out=rows["ar_row"][0:1, h * H:(h + 1) * H], in_=ps_ar[:])
    dq2 = dpool.tile([N], fp32, tag="dq2", name="dq2")
    nc.sync.dma_start(out=dq2[:].rearrange("(t p) -> p t", p=P), in_=sq2c[:])
    nc.sync.dma_start(out=rows["sq2_row"][:], in_=dq2[:][None, :])
    nc.vector.tensor_tensor(out=rows["ar_row"][:], in0=rows["ar_row"][:],
                            in1=rows["rinv_row"][:], op=AL.mult)
    nc.vector.scalar_tensor_tensor(
        out=rows["dflat_row"][:], in0=rows["ar_row"][:], scalar=1.0,
        in1=rows["sq2_row"][:],
        op0=AL.bypass, op1=AL.mult, accum_out=SCAL[0:1, 8:9])
    nc.vector.tensor_reduce(out=SCAL[:, 5:6], in_=sq2c[:],
                            axis=mybir.AxisListType.X, op=AL.add)
    # ss2 / F2sq
    ps_ss2 = psD.tile([K2, K2], fp32, space="PSUM", tag="ps_sm")
    for t in range(NT):
        nc.tensor.matmul(out=ps_ss2[:], lhsT=s2m[:, t * K2:(t + 1) * K2],
                         rhs=s2m[:, t * K2:(t + 1) * K2],
                         start=(t == 0), stop=(t == NT - 1))
    ss2_sb = uvpool.tile([K2, K2], fp32, tag="ss2")
    nc.scalar.copy(out=ss2_sb[:], in_=ps_ss2[:])
    ss2scr = uvpool.tile([K2, K2], fp32, tag="ss2scr")
    nc.vector.scalar_tensor_tensor(
        out=ss2scr[:], in0=ss2_sb[:], scalar=1.0, in1=ss2_sb[:],
        op0=AL.bypass, op1=AL.mult, accum_out=SCAL[0:K2, 6:7])
    # out_adj and pooled features
    ps_oa = psD.tile([K2, K2], fp32, space="PSUM", tag="ps_sm")
    for t in range(NT):
        nc.tensor.matmul(out=ps_oa[:], lhsT=s2m[:, t * K2:(t + 1) * K2],
                         rhs=vnm[:, t * K2:(t + 1) * K2],
                         start=(t == 0), stop=(t == NT - 1))
    oadj = uvpool.tile([K2, K2], fp32, tag="oadj")
    nc.scalar.copy(out=oadj[:], in_=ps_oa[:])
    ps_on = psD.tile([K2, HID], fp32, space="PSUM", tag="ps_sm")
    for t in range(NT):
        nc.tensor.matmul(out=ps_on[:], lhsT=s2m[:, t * K2:(t + 1) * K2],
                         rhs=h1nm[:, t * HID:(t + 1) * HID],
                         start=(t == 0), stop=(t == NT - 1))
    out_sb = uvpool.tile([K2, HID], fp32, tag="out_sb")
    nc.scalar.copy(out=out_sb[:], in_=ps_on[:])

    # ---------------- L: adjp normalization ----------------
    oadj2 = uvpool.tile([K2, K2], fp32, tag="oadj2")
    nc.vector.tensor_tensor(out=oadj2[:], in0=oadj[:], in1=mask16[:],
                            op=AL.mult)
    d2c = smpool.tile([K2, 1], fp32, tag="d2c")
    nc.vector.tensor_reduce(out=d2c[:], in_=oadj2[:],
                            axis=mybir.AxisListType.X, op=AL.add)
    sdc = smpool.tile([K2, 1], fp32, tag="sdc")
    nc.scalar.activation(out=sdc[:], in_=d2c[:], func=AF.Sqrt)
    sde = smpool.tile([K2, 1], fp32, tag="sde")
    nc.vector.tensor_scalar(out=sde[:], in0=sdc[:], scalar1=EPS, scalar2=0.0,
                            op0=AL.add, op1=AL.add)
    sdi = smpool.tile([K2, 1], fp32, tag="sdi")
    nc.vector.reciprocal(out=sdi[:], in_=sde[:])
    t_a = uvpool.tile([K2, K2], fp32, tag="t_a")
    nc.vector.tensor_scalar_mul(out=t_a[:], in0=oadj2[:], scalar1=sdi[:])
    ps_tp = psD.tile([K2, K2], fp32, space="PSUM", tag="ps_sm")
    nc.tensor.transpose(out=ps_tp[:], in_=t_a[:], identity=IDENT[0:K2, 0:K2])
    adjpT = uvpool.tile([K2, K2], fp32, tag="adjpT")
    nc.vector.tensor_scalar_mul(out=adjpT[:], in0=ps_tp[:], scalar1=sdi[:])

    # ---------------- M: conv2 + readout + mlp + log_softmax ----------------
    ps_ao = psD.tile([K2, HID], fp32, space="PSUM", tag="ps_sm")
    nc.tensor.matmul(out=ps_ao[:], lhsT=adjpT[:], rhs=out_sb[:],
                     start=True, stop=True)
    ao_sb = uvpool.tile([K2, HID], fp32, tag="ao_sb")
    nc.scalar.copy(out=ao_sb[:], in_=ps_ao[:])
    ps_aoT = psD.tile([HID, K2], fp32, space="PSUM", tag="ps_sm")
    nc.tensor.transpose(out=ps_aoT[:], in_=ao_sb[:], identity=IDENT[0:K2, 0:K2])
    aoTa = uvpool.tile([HID + 1, K2], fp32, tag="aoTa")
    nc.scalar.copy(out=aoTa[0:HID, :], in_=ps_aoT[:])
    nc.gpsimd.memset(aoTa[HID:HID + 1, :], 1.0)
    ps_oT = psD.tile([HID, K2], fp32, space="PSUM", tag="ps_sm")
    nc.tensor.transpose(out=ps_oT[:], in_=out_sb[:], identity=IDENT[0:K2, 0:K2])
    outT = uvpool.tile([HID, K2], fp32, tag="outT")
    nc.scalar.copy(out=outT[:], in_=ps_oT[:])
    ps_h2 = psD.tile([HID, K2], fp32, space="PSUM", tag="ps_sm")
    nc.tensor.matmul(out=ps_h2[:], lhsT=W["C2a"][:], rhs=aoTa[:],
                     start=True, stop=False)
    nc.tensor.matmul(out=ps_h2[:], lhsT=W["C2r"][:], rhs=outT[:],
                     start=False, stop=True)
    hsa = uvpool.tile([HID + 1, 1], fp32, tag="hsa")
    nc.vector.tensor_reduce(out=hsa[0:HID, :], in_=ps_h2[:],
                            axis=mybir.AxisListType.X, op=AL.add)
    nc.gpsimd.memset(hsa[HID:HID + 1, :], 1.0)
    ps_z = psD.tile([HID, 1], fp32, space="PSUM", tag="ps_sm")
    nc.tensor.matmul(out=ps_z[:], lhsT=W["L2a"][:], rhs=hsa[:],
                     start=True, stop=True)
    za = uvpool.tile([HID + 1, 1], fp32, tag="za")
    nc.scalar.activation(out=za[0:HID, :], in_=ps_z[:], func=AF.Relu)
    nc.gpsimd.memset(za[HID:HID + 1, :], 1.0)
    ps_lg = psD.tile([COUT, 1], fp32, space="PSUM", tag="ps_sm")
    nc.tensor.matmul(out=ps_lg[:], lhsT=W["L3a"][:], rhs=za[:],
                     start=True, stop=True)
    lg_col = uvpool.tile([COUT, 1], fp32, tag="lg_col")
    nc.scalar.copy(out=lg_col[:], in_=ps_lg[:])
    ps_lr = psD.tile([1, COUT], fp32, space="PSUM", tag="ps_sm")
    nc.tensor.transpose(out=ps_lr[:], in_=lg_col[:],
                        identity=IDENT[0:COUT, 0:COUT])
    lg_row = smpool.tile([1, COUT], fp32, tag="lg_row")
    nc.scalar.copy(out=lg_row[:], in_=ps_lr[:])
    mx = smpool.tile([1, 1], fp32, tag="mx")
    nc.vector.tensor_reduce(out=mx[:], in_=lg_row[:],
                            axis=mybir.AxisListType.X, op=AL.max)
    negm = smpool.tile([1, 1], fp32, tag="negm")
    nc.vector.tensor_scalar_mul(out=negm[:], in0=mx[:], scalar1=-1.0)
    erow = smpool.tile([1, COUT], fp32, tag="erow")
    se = smpool.tile([1, 1], fp32, tag="se")
    nc.scalar.activation(out=erow[:], in_=lg_row[:], func=AF.Exp,
                         bias=negm[:], accum_out=se[:])
    lnz = smpool.tile([1, 1], fp32, tag="lnz")
    nc.scalar.activation(out=lnz[:], in_=se[:], func=AF.Ln)
    mpl = smpool.tile([1, 1], fp32, tag="mpl")
    nc.vector.tensor_tensor(out=mpl[:], in0=mx[:], in1=lnz[:], op=AL.add)
    res_row = smpool.tile([1, COUT], fp32, tag="res_row")
    nc.vector.tensor_scalar(out=res_row[:], in0=lg_row[:], scalar1=mpl[:],
                            scalar2=0.0, op0=AL.subtract, op1=AL.add)
    nc.sync.dma_start(out=logits_d[g:g + 1, :], in_=res_row[:])

    # ---------------- N: reduce SCAL across partitions, write out ----------------
    ps_sc = psD.tile([1, NSCAL], fp32, space="PSUM", tag="ps_sm")
    nc.tensor.matmul(out=ps_sc[:], lhsT=ones_col[:], rhs=SCAL[:],
                     start=True, stop=True)
    scal_row = smpool.tile([1, NSCAL], fp32, tag="scal_row")
    nc.scalar.copy(out=scal_row[:], in_=ps_sc[:])
    nc.sync.dma_start(out=scal_d[g:g + 1, :], in_=scal_row[:])


def _prep_inputs(inputs, cap_chunks=None):
    import ml_dtypes
    bf16 = ml_dtypes.bfloat16

    x = np.asarray(inputs["x"], np.float32)
    ei = np.asarray(inputs["edge_index"])
    src = ei[0].astype(np.int64)
    dst = ei[1].astype(np.int64)

    g = src // N
    jt = (dst % N) // P
    u = (src % N) // P
    bucket = (g * NT + jt) * NT + u          # 0 .. B*64-1
    order = np.argsort(bucket, kind="stable")
    counts = np.bincount(bucket, minlength=B * NT * NT)
    need = int(np.ceil(max(1, counts.max()) / P))
    if cap_chunks is None:
        cap_chunks = max(3, need)
    assert need <= cap_chunks
    CAP = cap_chunks * P

    offs = np.zeros(B * NT * NT + 1, np.int64)
    np.cumsum(counts, out=offs[1:])
    sb = bucket[order]
    rank = np.arange(E, dtype=np.int64) - offs[sb]
    slot = sb * CAP + rank
    esrc_flat = np.full(B * NT * NT * CAP, -1.0, np.float32)
    edst_flat = np.full(B * NT * NT * CAP, -1.0, np.float32)
    esrc_flat[slot] = (src[order] % P).astype(np.float32)
    edst_flat[slot] = (dst[order] % P).astype(np.float32)

    per_core = []
    NB_CORE = GPC * NT * NT                  # buckets per core
    for c in range(NCORES):
        lo = c * NB_CORE * CAP
        hi = (c + 1) * NB_CORE * CAP
        es = esrc_flat[lo:hi].reshape(NB_CORE * cap_chunks, P).T
        ed = edst_flat[lo:hi].reshape(NB_CORE * cap_chunks, P).T
        xc = x[c * GPC * N:(c + 1) * GPC * N].reshape(GPC * NT, P, CIN)
        xt = np.ascontiguousarray(np.transpose(xc, (1, 0, 2)).reshape(P, GPC * NT * CIN))
        per_core.append({
            "xt": xt,
            "esrc": np.ascontiguousarray(es).astype(bf16),
            "edst": np.ascontiguousarray(ed).astype(bf16),
        })

    iota = np.broadcast_to(np.tile(np.arange(P, dtype=np.float32), 4), (P, 4 * P))
    iota = np.ascontiguousarray(iota).astype(bf16)

    def cat(wname, bname):
        return np.ascontiguousarray(np.concatenate(
            [np.asarray(inputs[wname], np.float32),
             np.asarray(inputs[bname], np.float32)[None]], 0))

    shared = {
        "iota": iota,
        "W1a": cat("lin1_w", "lin1_b"),
        "P1a": cat("pool1_w", "pool1_b"),
        "P2a": cat("pool2_w", "pool2_b"),
        "C1a": cat("conv1_rel_w", "conv1_rel_b"),
        "C1r": np.ascontiguousarray(np.asarray(inputs["conv1_root_w"], np.float32)),
        "C2a": cat("conv2_rel_w", "conv2_rel_b"),
        "C2r": np.ascontiguousarray(np.asarray(inputs["conv2_root_w"], np.float32)),
        "L2a": cat("lin2_w", "lin2_b"),
        "L3a": cat("lin3_w", "lin3_b"),
    }
    for m in per_core:
        m.update(shared)
    eg = counts.reshape(B, NT * NT).sum(1).astype(np.float64)
    return per_core, cap_chunks, eg


def _combine(logits_list, scal_list, eg):
    logits = np.concatenate(logits_list, 0)
    scal = np.concatenate(scal_list, 0).astype(np.float64)
    sds, dinsq, sAD2 = scal[:, 0], scal[:, 1], scal[:, 2]
    # remove the EPS_POS sqrt-shift contribution (counts * EPS_POS) exactly
    sAD2 = sAD2 - EPS_POS * eg
    T1, F1sq = scal[:, 3], scal[:, 4]
    T2, F2sq = scal[:, 5], scal[:, 6]
    num, den = scal[:, 7], scal[:, 8]
    sas = 0.5 * (sds + dinsq - sAD2)
    ct = (sds - sas) / (sds + EPS)
    o1 = np.sqrt(np.maximum(2.0 - 2.0 * T1 / (np.sqrt(F1sq) * np.sqrt(K1)), 0.0))
    mc = num / (den + EPS)
    o2 = np.sqrt(np.maximum(2.0 - 2.0 * T2 / (np.sqrt(F2sq) * np.sqrt(K2)), 0.0))
    loss1 = np.float32(ct.mean() + o1.mean())
    loss2 = np.float32(-mc.mean() + o2.mean())
    return logits.astype(np.float32), loss1, loss2


def kernel(**inputs):
    from concourse.bass_utils import run_bass_kernel_spmd

    per_core, cap_chunks, eg = _prep_inputs(inputs)
    key = cap_chunks
    if key not in _CACHE:
        _CACHE[key] = _build_nc(cap_chunks)
    nc = _CACHE[key]

    res = run_bass_kernel_spmd(nc, per_core, core_ids=list(range(NCORES)))
    logits_list = [r["logits"] for r in res.results]
    scal_list = [r["scal"] for r in res.results]
    return _combine(logits_list, scal_list, eg)


if __name__ == "__main__":
    import reference
    inputs = reference.setup_inputs()
    out = kernel(**{k: np.asarray(v) for k, v in inputs.items()})
    print("logits[0]:", out[0][0])
    print("loss1:", out[1], "loss2:", out[2])
